# revision 29
# baseline (speedup 1.0000x reference)
"""Trainium2 Bass kernel for batched tanh-attention flat-softmax.

Per batch b:
    Q = query[b] @ W_query; K = query[b] @ W_key      # [S, 64]
    s = tanh(Q @ K.T) * 10                            # [S, S]
    s[diag] = -inf
    out[b] = softmax(s.flatten())

Sharding: data-parallel over batch across 8 NeuronCores (6 batches/core),
W_query/W_key replicated; no cross-core communication.

Device computes W = exp(10*tanh(s)) (fp16) and per-batch accumulator sums
Z_dev = sum(W).  The host finishes the softmax during the mandatory
fp16->fp32 upcast: out = W * 1/(Z_dev - trace(W)), diagonal zeroed (the
reference's -1e8 diagonal mask makes those entries exactly 0 in fp32;
removing trace(W) from Z is the same correction).

The ScalarE (ACT) engine is the hard bottleneck (1 elem/cycle @1.2GHz, and
tanh+exp both need it).  To break the ACT floor, 3 of the 8 row-chunks per
batch compute exp on the *Vector* engine instead, via two custom DVE ops:

    exp(10*t) = (p(t))^32,  p(t) = 1 + c1 t + c2 t^2 + c3 t^3 ~ e^{0.3125 t}
    pass1: g4 = p(t)^4   (Horner + 2 squarings, depth 8, fp32 out)
    pass2: w  = g4^8     (3 squarings + ADD accumulation, fp16 out)

p is constrained to p(0)=1 so the DVE chunks carry the exact same scale as
the ACT-exp chunks (softmax normalization cancels any common factor; a
free constant would NOT cancel across mixed chunks).  Max rel error of the
DVE path ~5e-3 (poly ^32 ~2.2e-3 + fp16 tanh storage), same order as the
ACT path's fp16 quantization.

The PSUM->bf16 projection cast runs on the Pool (gpsimd) engine to keep
the Vector engine free for the exp chunks.
"""

import numpy as np
import ml_dtypes

import concourse.bass as bass
import concourse.mybir as mybir
import concourse.tile as tile
from concourse import bacc
from concourse.bass_utils import run_bass_kernel_spmd

import concourse.dve_ops as dve_ops
from concourse.dve_spec import (
    AluOp, C0, C1, C2, One, Spec, Src0, _has_src1, lower, sq,
)
from concourse.dve_uop import DveOpSpec

B = 48
S = 1024
D = 128
DK = 64
N_CORES = 8
BPC = B // N_CORES
P = 128
NQ = S // P
NDV = 3          # chunks per batch exp'd on the Vector engine (rest: ACT)
F32 = mybir.dt.float32
F16 = mybir.dt.float16
BF16 = mybir.dt.bfloat16
AL = mybir.AluOpType

TANH_CLIP = 10.0
# cubic fit of e^{0.3125 t} on [-1,1] with p(0)=1 (minimax relative)
EXP_C1 = 0.3125404800
EXP_C2 = 0.0491554200
EXP_C3 = 0.0050490700


def _register_dve_ops():
    """Append the two exp custom-DVE ops to the dve_ops registry (documented
    extension point: new ops are appended, rows assigned positionally)."""
    existing = {op.name: op for op in dve_ops.OPS}
    if "EXP10T_P1" in existing:
        return existing["EXP10T_P1"], existing["EXP10T_P2"]

    spec1 = Spec(
        body=sq(sq(One + Src0 * (C0 + Src0 * (C1 + Src0 * C2)))),
        reference=lambda in0, s0, s1, imm2:
            (1.0 + in0 * (s0 + in0 * (s1 + in0 * imm2))) ** 4,
    )
    spec2 = Spec(
        body=sq(sq(sq(Src0))),
        accum=AluOp.ADD,
        reference=lambda in0, s0, s1, imm2: in0 ** 8,
    )
    out = []
    for name, spec in (("EXP10T_P1", spec1), ("EXP10T_P2", spec2)):
        row = dve_ops._CUSTOM_DVE_ROW_BASE + len(dve_ops.OPS)
        assert row < 0x20
        shas = {}
        for ver in ("v3", "v4"):
            shas[ver] = DveOpSpec(
                name=name, opcode=row, uops=lower(spec, ver=ver),
                rd1_en=_has_src1(spec),
            ).sha(ver)
        op = dve_ops.DveOp(name, spec, subdim=False, uops_sha=shas)
        dve_ops.OPS.append(op)
        dve_ops.CUSTOM_DVE_SPECS[name] = spec
        dve_ops._SUB_OPCODE_FOR_NAME[name] = row
        out.append(op)
    return out[0], out[1]


EXP10T_P1, EXP10T_P2 = _register_dve_ops()


def build_bass() -> bass.Bass:
    nc = bacc.Bacc(None, target_bir_lowering=False)

    qh_d = nc.dram_tensor("query", [BPC, S, D], BF16, kind="ExternalInput")
    # weight stacks prepared on host, transposed: rows of [whA;whB].T so one
    # xbar transpose (same DMA mode as the query loads) lands them in
    # [d, col] layout
    wst_d = nc.dram_tensor("wstackT", [2 * P, D], BF16, kind="ExternalInput")
    out_d = nc.dram_tensor("out", [BPC, S, S], F16, kind="ExternalOutput")
    z_d = nc.dram_tensor("z", [P, 2 * BPC + 1], F32, kind="ExternalOutput")

    with tile.TileContext(nc) as tc:
        with (
            tc.tile_pool(name="singles", bufs=1) as singles,
            tc.tile_pool(name="qtp", bufs=2) as qtp,
            tc.tile_pool(name="hbp", bufs=2) as hbp,
            tc.tile_pool(name="tbuf", bufs=3) as tbuf,
            tc.tile_pool(name="gbuf", bufs=2) as gbuf,
            tc.tile_pool(name="ps", bufs=2, space="PSUM") as psp,
        ):
            # --- one-time setup ---
            # weights first (small), then query[0] as two half-column
            # transposes so proj h=0 can start before the second half lands
            wsb = singles.tile([D, 2 * P], BF16)
            nc.sync.dma_start_transpose(wsb, wst_d[:, :])
            whA, whB = wsb[:, 0:P], wsb[:, P:2 * P]

            qhT0 = qtp.tile([D, S], BF16, tag="qhT")
            nc.sync.dma_start_transpose(qhT0[:, 0:512], qh_d[0][0:512])
            nc.sync.dma_start_transpose(qhT0[:, 512:S], qh_d[0][512:S])

            # accumulator sums: column b = ACT accum, column BPC+b = DVE
            # accum, column 2*BPC = the last batch's split second ACT piece
            zrow = singles.tile([P, 2 * BPC + 1], F32)

            def load_q(b):
                """DMA-transpose query[b] (bf16) straight from DRAM."""
                qhT = qtp.tile([D, S], BF16, tag="qhT")
                nc.sync.dma_start_transpose(qhT, qh_d[b])
                return qhT

            def proj(qhT):
                """pp[:,0] = A = [Q;K], pp[:,1] = B = [K;Q] (fp32 psum).
                Column-half-major order so the cast (and the first scores
                matmuls) can start after half the projection."""
                pp = psp.tile([P, 2, S], F32, tag="ps", name="pp")
                for h in range(2):
                    cols = slice(h * 512, (h + 1) * 512)
                    for w, half in ((whA, 0), (whB, 1)):
                        nc.tensor.matmul(
                            pp[:, half, cols], w, qhT[:, cols],
                            start=True, stop=True,
                        )
                return pp

            def cast_hb(pp):
                hb = hbp.tile([P, 2, S], BF16, tag="hb")
                for h in range(2):
                    cols = slice(h * 512, (h + 1) * 512)
                    nc.vector.tensor_copy(hb[:, :, cols], pp[:, :, cols])
                return hb

            def scores_pair(t_sb, hb, j):
                """Two 128-row score chunks (qc=2j, 2j+1) in one 4-bank PSUM
                tile; the two 64-contraction matmuls stream CONCURRENTLY in
                different PE row groups. One tanh (no diag handling -- the
                host zeroes the diagonal and corrects Z by trace(W))."""
                sc = psp.tile([P, 2, S], F32, tag="ps", name=f"sc{j}")
                sl0 = slice((2 * j) * P, (2 * j + 1) * P)
                sl1 = slice((2 * j + 1) * P, (2 * j + 2) * P)
                A, Bv = hb[:, 0], hb[:, 1]
                for h in range(2):
                    cols = slice(h * 512, (h + 1) * 512)
                    nc.tensor.matmul(
                        sc[:, 0, cols], A[0:DK, sl0], Bv[0:DK, cols],
                        start=True, stop=True, tile_position=(0, 0),
                    )
                    nc.tensor.matmul(
                        sc[:, 1, cols], Bv[DK:P, sl1], A[DK:P, cols],
                        start=True, stop=True, tile_position=(DK, 0),
                    )
                nc.scalar.activation(
                    out=t_sb[:, 2 * j:2 * j + 2],
                    in_=sc,
                    func=mybir.ActivationFunctionType.Tanh,
                )

            def dve_exp_p1(t_sb, g4, sl):
                """pass1: g4 = p(t)^4 (fp32) for chunk range sl."""
                nc.vector._custom_dve(
                    EXP10T_P1, out=g4[:, sl], in0=t_sb[:, sl],
                    s0=EXP_C1, s1=EXP_C2, imm2=EXP_C3,
                )

            def dve_exp_p2(t_sb, g4, b):
                """pass2: w = g4^8 (fp16, in place over chunks 0:NDV) with
                fp32 ADD accumulation."""
                nc.vector._custom_dve(
                    EXP10T_P2, out=t_sb[:, 0:NDV], in0=g4[:, 0:NDV],
                    accum_out=zrow[:, BPC + b:BPC + b + 1],
                )

            def exp_act(t_sb, b, sl, zcol):
                """ACT exp(10*t) in place (fp16) over chunk range sl, fp32
                accumulator -> zrow[:, zcol]."""
                nc.scalar.activation(
                    out=t_sb[:, sl],
                    in_=t_sb[:, sl],
                    func=mybir.ActivationFunctionType.Exp,
                    scale=TANH_CLIP,
                    accum_out=zrow[:, zcol:zcol + 1],
                )

            def store(b, t_sb, sl):
                """Store chunk range sl of batch b (fp16, unnormalized).
                SWDGE ring so the sync ring keeps xbar mode for transposes."""
                nc.gpsimd.dma_start(
                    out_d[b].rearrange("(n p) s -> p n s", p=P)[:, sl],
                    t_sb[:, sl],
                )

            # ---- software-pipelined batch loop --------------------------
            hb = cast_hb(proj(qhT0))

            for b in range(BPC):
                t_sb = tbuf.tile([P, NQ, S], F16, tag="t")
                g4 = gbuf.tile([P, NDV, S], F32, tag="g4")

                if b + 1 < BPC:
                    nqhT = load_q(b + 1)

                scores_pair(t_sb, hb, 0)            # chunks 0,1
                dve_exp_p1(t_sb, g4, slice(0, 2))   # DVE pass1 on 0:2
                scores_pair(t_sb, hb, 1)            # chunks 2,3
                dve_exp_p1(t_sb, g4, slice(2, NDV))  # DVE pass1 on 2:3
                scores_pair(t_sb, hb, 2)            # chunks 4,5
                # sc3 allocates BEFORE pp(b+1) so its PSUM slot waits on
                # tanh1 (early) instead of tanh2; proj then fills the PE
                # during tanh3/exp.  The cast is issued BEFORE pass2 on the
                # DVE queue: pass2 only feeds the store+accum (off the
                # critical path), while the cast gates next batch's pair0.
                scores_pair(t_sb, hb, 3)            # chunks 6,7
                if b + 1 < BPC:
                    nhb = cast_hb(proj(nqhT))
                dve_exp_p2(t_sb, g4, b)             # DVE pass2 on 0:NDV
                store(b, t_sb, slice(0, NDV))       # DVE chunks out early
                if b + 1 < BPC:
                    exp_act(t_sb, b, slice(NDV, NQ), b)
                    store(b, t_sb, slice(NDV, NQ))
                    hb = nhb
                else:
                    # last batch: split the tail so the final store is small
                    exp_act(t_sb, b, slice(NDV, 6), b)
                    store(b, t_sb, slice(NDV, 6))
                    exp_act(t_sb, b, slice(6, NQ), 2 * BPC)
                    store(b, t_sb, slice(6, NQ))

            # epilogue: the Z matrix rides the (now idle) sync ring
            nc.sync.dma_start(z_d[:, :], zrow)

    nc.compile()
    return nc


_CACHED_NC = None


def make_in_maps(inputs) -> list:
    """Host-side input marshalling: bf16 query + bf16 hi/lo weight stacks."""
    query = np.asarray(inputs["query"], dtype=np.float32)
    wq = np.asarray(inputs["W_query"], dtype=np.float32)
    wk = np.asarray(inputs["W_key"], dtype=np.float32)
    assert query.shape == (B, S, D), query.shape
    qh = np.ascontiguousarray(query.astype(ml_dtypes.bfloat16))

    wA = np.concatenate([wq, wk], axis=1)          # [D, 2*DK]
    wB = np.concatenate([wk, wq], axis=1)
    whA = wA.astype(ml_dtypes.bfloat16)
    whB = wB.astype(ml_dtypes.bfloat16)
    # transposed stack: one xbar DMA-transpose lands [whA|whB] in
    # [d, col] layout on device
    wstackT = np.ascontiguousarray(np.vstack([whA.T, whB.T]))
    return [
        {"query": qh[c * BPC:(c + 1) * BPC], "wstackT": wstackT}
        for c in range(N_CORES)
    ]


def kernel(**inputs: np.ndarray) -> np.ndarray:
    global _CACHED_NC
    if _CACHED_NC is None:
        _CACHED_NC = build_bass()
    nc = _CACHED_NC

    in_maps = make_in_maps(inputs)
    res = run_bass_kernel_spmd(nc, in_maps, core_ids=list(range(N_CORES)))

    out = np.empty((B, S * S), dtype=np.float32)
    idx = np.arange(S)
    for c, r in enumerate(res.results):
        w = r["out"]                      # [BPC, S, S] fp16, unnormalized
        z = r["z"].astype(np.float64)     # [P, 2*BPC+1]
        for b in range(BPC):
            wb = w[b]
            tr = wb.diagonal().astype(np.float64).sum()
            zb = z[:, b].sum() + z[:, BPC + b].sum()
            if b == BPC - 1:
                zb += z[:, 2 * BPC].sum()
            rz = np.float32(1.0 / (zb - tr))
            ob = wb.astype(np.float32)
            ob *= rz
            ob[idx, idx] = 0.0
            out[c * BPC + b] = ob.reshape(S * S)
    return out
